# revision 5
# baseline (speedup 1.0000x reference)
"""MiniRocket feature extraction kernel for Trainium2 (8 NeuronCores, data parallel).

v2: PSUM-direct counting. The baseline evicted resp PSUM->SBUF fp16 on ACT and
counted on DVE(is_gt)/ACT(Sign); measured HW shows DVE runs these at 1x
(1.04 ns/col) regardless of dtype, so the fp16 eviction bought nothing. Here:

  - resp [84, 2048] f32 stays in PSUM; counting ops read PSUM directly.
  - The odd-parity edge "poison" is folded into the matmul: patch row 72 is an
    edge-indicator row (1.0 on [0,pad) u [L-pad,L)) and W row 72 is -10000 for
    the 42 odd-parity kernels, so their edge responses can never exceed a bias.
    No per-iteration poison op anywhere.
  - Patch DMAs ride the SP HWDGE queue (SP sequencer is otherwise idle).
  - Counting is split DVE/ACT by a static schedule (l-ranges per feature)
    balanced against measured per-engine rates.
  - Counts accumulate into per-engine tables cnt_dve/cnt_act [84, 128]
    (col = b*16 + d*4 + f); one batched affine tail produces the output.

walrus encodes at most ONE sync wait per compute/DMA instruction;
_legalize_sync_waits (rules as in the baseline: self-wait drop, Ldweights
hoist, tail-drain pruning, same-sem max-collapse) legalizes Tile's waits.
"""

import os
import sys

for _p in (
    "/root/.axon_site",
    "/root/.axon_site/_ro/trn_rl_repo",
    "/root/.axon_site/_ro/pypackages",
    "/opt/trn_rl_repo",
):
    if os.path.isdir(_p) and _p not in sys.path:
        sys.path.append(_p)

import numpy as np

B, L, C = 64, 2048, 8
DILATIONS = (1, 2, 4, 8)
D = 4
K = 84
F = 4
KERNEL_LEN = 9
NCORES = 8
BPC = B // NCORES
PAD = 32
LP = L + 2 * PAD
POISON = -10000.0
BIG = 1.0e30
SAT8 = 448.0  # float8e4 (e4m3) saturation value

# Counting schedule: per (d) iteration, ops as (engine, f, lo, hi).
# 'dve' = tensor_scalar is_gt + accum (raw = count over [lo,hi))
# 'act' = Sign + accum (raw = 2*count - len, zeros counting 0)
# Measured HW rates: DVE accum-op ~2341ns per 2048 cols (accum_out disables
# the DVE fast modes); ACT Sign ~2000+185ns. Balanced split: DVE f0+f1,
# ACT f2+f3, with X1 trimming f1's tail over to ACT if < L.
ACT_MODE = "sign"
X1 = L  # DVE takes f0 fully + [0, X1) of f1; ACT takes the rest.

_PROGRAM_CACHE: dict = {}


def _schedule():
    ops = []
    ops.append(("dve", 0, 0, L))
    if X1 > 0:
        ops.append(("dve", 1, 0, X1))
    if X1 < L:
        ops.append(("act", 1, X1, L))
    ops.append(("act", 2, 0, L))
    ops.append(("act", 3, 0, L))
    return ops


def _parity_perm(d_idx: int) -> np.ndarray:
    k = np.arange(K)
    parity = (d_idx + k) % 2
    return np.concatenate([k[parity == 1], k[parity == 0]])


def _host_constants(kernels, channel_masks, bias_matrices, feature_mean, feature_std):
    """Build wT [73, 4*84] f16, eT [4, L] f16, cpk [84, 416] f32, perms."""
    kernels = np.asarray(kernels, np.float32)
    channel_masks = np.asarray(channel_masks, np.float32)
    bias_matrices = np.asarray(bias_matrices, np.float32)
    feature_mean = np.asarray(feature_mean, np.float32).reshape(D, K, F)
    feature_std = np.asarray(feature_std, np.float32).reshape(D, K, F)

    sched = _schedule()

    wT_blocks = []
    eT = np.zeros((D, L), np.float16)
    bias_dve = np.zeros((K, 16), np.float32)
    nbias_act = np.zeros((K, 16), np.float32)
    Ad = np.zeros((K, 128), np.float32)
    Aa = np.zeros((K, 128), np.float32)
    Bt = np.zeros((K, 128), np.float32)
    perms = []
    for d_idx, dil in enumerate(DILATIONS):
        perm = _parity_perm(d_idx)
        perms.append(perm)
        w = channel_masks[d_idx][perm][:, :, None] * kernels[perm][:, None, :]
        w73 = np.zeros((73, K), np.float32)
        w73[:72] = w.reshape(K, C * KERNEL_LEN).T
        w73[72, 0:42] = POISON  # odd-parity (trimmed) kernels come first
        wT_blocks.append(w73.astype(np.float16))

        pad = 4 * dil
        eT[d_idx, 0:pad] = 1.0
        eT[d_idx, L - pad : L] = 1.0

        w_sel = np.where(np.arange(K) < 42, 1.0 / (L - 2 * pad), 1.0 / L).astype(
            np.float32
        )
        bias_p = bias_matrices[d_idx][perm]  # [84, 4]
        mean_p = feature_mean[d_idx][perm]
        std_p = feature_std[d_idx][perm]
        A = w_sel[:, None] / std_p  # [84, 4]
        B0 = -mean_p / std_p  # [84, 4]

        for f in range(F):
            bias_dve[:, 4 * d_idx + f] = bias_p[:, f]
            nbias_act[:, 4 * d_idx + f] = (
                -BIG * bias_p[:, f] if ACT_MODE == "sat8" else -bias_p[:, f]
            )
        for b in range(BPC):
            for f in range(F):
                col = b * 16 + 4 * d_idx + f
                Bt[:, col] = B0[:, f]
                for eng, ff, lo, hi in sched:
                    if ff != f:
                        continue
                    if eng == "dve":
                        Ad[:, col] = A[:, f]
                    else:
                        if ACT_MODE == "sat8":
                            Aa[:, col] = A[:, f] / (2.0 * SAT8)
                        else:
                            Aa[:, col] = A[:, f] / 2.0
                        Bt[:, col] += A[:, f] * (hi - lo) / 2.0

    wT = np.concatenate(wT_blocks, axis=1)  # [73, 336]
    cpk = np.concatenate([bias_dve, nbias_act, Ad, Aa, Bt], axis=1)  # [84, 416]
    return wT[:72], wT[72:73].copy(), cpk, perms


def _build_program():
    from contextlib import ExitStack

    import bass_rust
    import concourse.bass as bass
    import concourse.tile as tile
    from concourse import mybir

    def shifted_ap(center_slice, dil):
        c = center_slice.copy()
        c.offset = c.offset - 4 * dil
        c.ap = bass_rust.VecI64Pair([[LP, C], [dil, KERNEL_LEN], [1, L]])
        return c

    f16 = mybir.dt.float16
    f32 = mybir.dt.float32
    f8 = mybir.dt.float8e4
    A_ = mybir.AluOpType
    AF = mybir.ActivationFunctionType

    sched = _schedule()

    nc = bass.Bass()
    xT = nc.declare_dram_parameter("xT", [BPC * C, LP], f16, isOutput=False)
    wT = nc.declare_dram_parameter("wT", [72, D * K], f16, isOutput=False)
    wP = nc.declare_dram_parameter("wP", [1, D * K], f16, isOutput=False)
    cpk = nc.declare_dram_parameter("cpk", [K, 416], f32, isOutput=False)
    out = nc.declare_dram_parameter("out", [BPC, K, 16], f32, isOutput=True)

    with tile.TileContext(nc) as tc, ExitStack() as ctx:
        cpool = ctx.enter_context(tc.tile_pool(name="const", bufs=1))
        patch_pool = ctx.enter_context(tc.tile_pool(name="patch", bufs=BPC * D))
        psum_pool = ctx.enter_context(tc.tile_pool(name="psum", bufs=2, space="PSUM"))
        tr_pool = ctx.enter_context(tc.tile_pool(name="tr", bufs=1))
        cnt_pool = ctx.enter_context(tc.tile_pool(name="cnt", bufs=1))
        osb_pool = ctx.enter_context(tc.tile_pool(name="osb", bufs=1))
        scr_pool = ctx.enter_context(tc.tile_pool(name="scr", bufs=1))

        xsb = cpool.tile([BPC * C, LP], f16)
        nc.sync.dma_start(xsb[:], xT.ap())
        wsb = cpool.tile([72, D * K], f16)
        nc.sync.dma_start(wsb[:], wT.ap())
        wpois = cpool.tile([1, D * K], f16)
        nc.sync.dma_start(wpois[:], wP.ap())
        csb = cpool.tile([K, 416], f32)
        nc.sync.dma_start(csb[:], cpk.ap())

        trash_v = tr_pool.tile([K, L], f16)
        trash_a = tr_pool.tile([K, L], f8 if ACT_MODE == "sat8" else f16)
        cnt_dve = cnt_pool.tile([K, 128], f32)
        cnt_act = cnt_pool.tile([K, 128], f32)
        m1 = osb_pool.tile([K, 128], f32)
        m2 = osb_pool.tile([K, 128], f32)
        osb = osb_pool.tile([K, BPC * 16], f32)
        scr = scr_pool.tile([1, 72], f32)

        # zero the count tables on their own engines (engine-local FIFO later)
        nc.vector.memset(cnt_dve[:], 0.0)
        nc.scalar.memzero(cnt_act[:])

        ones = cpool.tile([1, 64], f16)
        nc.gpsimd.memset(ones[:], 1.0)

        # prime each engine's vector clock with the csb DMA tick; also pull
        # DVE's startup ticks into ACT's clock
        nc.vector.tensor_copy(m1[:, 0:1], csb[:, 0:1])
        nc.scalar.activation(scr[0:1, 0:1], csb[0:1, 0:1], AF.Copy)
        nc.scalar.activation(scr[0:1, 1:2], m1[0:1, 0:1], AF.Copy)

        pend_gate = None
        for b in range(BPC):
            for d_idx, dil in enumerate(DILATIONS):
                it = b * D + d_idx
                pad = 4 * dil
                # Gate + closer for the PREVIOUS iteration (both tiny, on
                # ACT): the gate reads iter N-1's last DVE accum column
                # (write-once -> no WAR back-edge), pulling DVE-(N-1) into
                # ACT's vector clock; the closer then reads a byte of
                # ps-(N-1), becoming its last PSUM reader on ACT with a clock
                # covering the DVE readers. The legalizer drops DVE_ waits
                # from matmuls that wait on the closer.
                if pend_gate is not None:
                    pit, pgcol, pps = pend_gate
                    nc.scalar.activation(
                        scr[0:1, 2 + 2 * pit : 3 + 2 * pit],
                        cnt_dve[0:1, pgcol : pgcol + 1],
                        AF.Copy,
                    )
                    nc.scalar.activation(
                        scr[0:1, 3 + 2 * pit : 4 + 2 * pit], pps[0:1, 0:1], AF.Copy
                    )
                patch = patch_pool.tile([72, L], f16)
                nc.sync.dma_start(
                    patch[0:72, 0:L],
                    shifted_ap(xsb[C * b : C * b + C, PAD : PAD + L], dil),
                )
                ps = psum_pool.tile([K, L], f32)
                for nt in range(4):
                    nc.tensor.matmul(
                        ps[:, nt * 512 : (nt + 1) * 512],
                        lhsT=wsb[:, d_idx * K : (d_idx + 1) * K],
                        rhs=patch[:, nt * 512 : (nt + 1) * 512],
                        start=True,
                        stop=True,
                    )
                # edge poison: accumulate W row 72 (-10000 on the 42 trimmed
                # kernels) against a ones row over the pad-wide edge columns,
                # so those responses can never exceed a bias.
                for elo, ehi in ((0, pad), (L - pad, L)):
                    nc.tensor.matmul(
                        ps[:, elo:ehi],
                        lhsT=wpois[0:1, d_idx * K : (d_idx + 1) * K],
                        rhs=ones[0:1, 0 : ehi - elo],
                        start=False,
                        stop=True,
                        skip_group_check=True,
                    )
                for eng, f, lo, hi in sched:
                    col = b * 16 + 4 * d_idx + f
                    if eng == "dve":
                        nc.vector.tensor_scalar(
                            trash_v[:, lo:hi],
                            ps[:, lo:hi],
                            csb[:, 4 * d_idx + f : 4 * d_idx + f + 1],
                            None,
                            A_.is_gt,
                            A_.add,
                            accum_out=cnt_dve[:, col : col + 1],
                        )
                    elif ACT_MODE == "sat8":
                        nc.scalar.activation(
                            trash_a[:, lo:hi],
                            ps[:, lo:hi],
                            AF.Identity,
                            bias=csb[:, 16 + 4 * d_idx + f : 16 + 4 * d_idx + f + 1],
                            scale=BIG,
                            accum_out=cnt_act[:, col : col + 1],
                        )
                    else:
                        nc.scalar.activation(
                            trash_a[:, lo:hi],
                            ps[:, lo:hi],
                            AF.Sign,
                            bias=csb[:, 16 + 4 * d_idx + f : 16 + 4 * d_idx + f + 1],
                            accum_out=cnt_act[:, col : col + 1],
                        )
                # Remember this iteration's gate/closer args; they are
                # emitted at the TOP of the next iteration (software
                # pipelining) so the gate's wait on the slow DVE f1 op never
                # FIFO-blocks the next iteration's signs.
                gcol = b * 16 + 4 * d_idx + 1  # f1 accum column (last DVE op)
                pend_gate = (it, gcol, ps)

        # tail: osb = cnt_dve*Ad + cnt_act*Aa + Bt  (all [84, 128])
        nc.vector.tensor_tensor(m1[:], cnt_dve[:], csb[:, 32:160], A_.mult)
        nc.vector.tensor_tensor(m2[:], cnt_act[:], csb[:, 160:288], A_.mult)
        nc.vector.tensor_tensor(m1[:], m1[:], m2[:], A_.add)
        nc.vector.tensor_tensor(osb[:], m1[:], csb[:, 288:416], A_.add)

        dst = out.ap().copy()
        dst.ap = bass_rust.VecI64Pair([[16, K], [K * 16, BPC], [1, 16]])
        nc.sync.dma_start(dst, osb[:])

    _legalize_sync_waits(nc, bass_rust)
    return nc


_FIFO_SELF_SEM = {
    "Matmult": "PE_",
    "Ldweights": "PE_",
    "Activation": "Activation_",
    "TensorScalarPtr": "DVE_",
    "TensorTensor": "DVE_",
    "TensorReduce": "DVE_",
    "Memset": None,
}


def _legalize_sync_waits(nc, bass_rust):
    """walrus encodes at most ONE sync wait per compute/DMA instruction.
    Rules (validated in CoreSim + hardware by the baseline kernel):
     0. Collapse multiple waits on the same semaphore to the max tick.
     1. Drop same-engine self-waits when an instruction holds other waits.
     2. Hoist extra Matmult waits onto the immediately-preceding Ldweights.
     3. Prune the kernel-tail SP drain (see baseline docstring).
    """
    blocks = list(nc.m.functions[0].blocks)
    end_blk = next(b for b in blocks if b.name.endswith("_end"))

    max_waited: dict = {}
    for blk in blocks:
        for inst in blk.instructions:
            if blk is end_blk:
                continue
            si = inst.sync_info
            for w in si.on_wait if si and si.on_wait else []:
                if w.wait_value > max_waited.get(w.ant_name, -1):
                    max_waited[w.ant_name] = w.wait_value

    def _tensor_of(arg):
        try:
            return arg.memsetref or ""
        except AttributeError:
            return ""

    # patch-feeding DMAs (dst tile "patch*", src "xsb*"), in iteration order
    patch_dmas = []
    for blk in blocks:
        for inst in blk.instructions:
            if inst.opcode == "DMACopy" and inst.outs and inst.ins:
                if _tensor_of(inst.outs[0]).startswith("patch") and _tensor_of(
                    inst.ins[0]
                ).startswith("xsb"):
                    patch_dmas.append(inst)
    for blk in blocks:
        prev = None
        proc: list = []  # processed instructions in this block, in order
        for inst in blk.instructions:
            si = inst.sync_info
            if si is None or not si.on_wait:
                prev = inst
                proc.append(inst)
                continue
            waits = list(si.on_wait)
            # rule 0: same-sem collapse
            best: dict = {}
            for w in waits:
                if w.ant_name not in best or w.wait_value > best[w.ant_name].wait_value:
                    best[w.ant_name] = w
            waits = list(best.values())
            pfx = _FIFO_SELF_SEM.get(inst.opcode)
            if pfx and len(waits) > 1:
                waits = [w for w in waits if not w.ant_name.startswith(pfx)]
            if inst.opcode == "Matmult" and len(waits) > 1:
                # The ACT wait is the closer op whose vector clock (via the
                # gate) covers the DVE readers of the PSUM slot.
                if any(w.ant_name.startswith("Activation_") for w in waits):
                    waits = [w for w in waits if not w.ant_name.startswith("DVE_")]
            if inst.opcode == "DMACopy" and len(waits) > 1:
                # queue FIFO already orders DMAs on the same queue
                non_q = [w for w in waits if not w.ant_name.startswith("DMAHW")]
                if non_q:
                    waits = non_q
            if (
                inst.opcode in ("Activation", "TensorScalarPtr", "TensorTensor")
                and len(waits) > 1
            ):
                # backward-deposit: park excess waits on earlier same-engine
                # instructions with empty wait sets (engine FIFO preserves the
                # ordering; producers of these waits are upstream of this
                # engine's stream, so no cycles).
                eng = inst.engine
                for pi in reversed(proc):
                    if len(waits) <= 1:
                        break
                    if pi.engine != eng or pi.opcode in ("Drain",):
                        continue
                    psi2 = pi.sync_info
                    if psi2 is not None and psi2.on_wait:
                        continue
                    if psi2 is None:
                        psi2 = bass_rust.SyncInfo(on_wait=[], on_update=[])
                        pi.sync_info = psi2
                    psi2.on_wait = [waits.pop(0)]
                assert len(waits) <= 1, (
                    f"{inst.opcode} {inst.name}: could not legalize "
                    f"{len(waits)} waits"
                )
            if inst.opcode == "Matmult" and len(waits) > 1:
                assert prev is not None and prev.opcode == "Ldweights", (
                    f"matmul {inst.name} has {len(waits)} waits and no "
                    f"preceding Ldweights (prev={prev and prev.opcode})"
                )
                psi = prev.sync_info
                if psi is None:
                    psi = bass_rust.SyncInfo(on_wait=[], on_update=[])
                    prev.sync_info = psi
                psi.on_wait = list(psi.on_wait) + waits[:-1]
                waits = waits[-1:]
            si.on_wait = waits
            prev = inst
            proc.append(inst)

    end_insts = list(end_blk.instructions)
    tail = end_insts[0]
    assert tail.opcode == "Drain", f"unexpected end block head {tail.opcode}"
    si = tail.sync_info
    if si and len(si.on_wait) > 1:
        eng_pfx = ("Activation_", "PE_", "DVE_", "Pool_", "SP_")
        keep = [
            w
            for w in si.on_wait
            if not w.ant_name.startswith(eng_pfx)
            and max_waited.get(w.ant_name, -1) < w.wait_value
        ]
        if len(keep) > 1:
            spill_slots = []
            for inst in end_insts[1:]:
                if inst.opcode == "ISA":
                    break
                isi = inst.sync_info
                if inst.opcode == "Drain" and (not isi or not isi.on_wait):
                    spill_slots.append(inst)
            assert len(spill_slots) >= len(keep) - 1, (
                f"tail drain needs {len(keep)} wait slots, "
                f"only {1 + len(spill_slots)} available"
            )
            for w, slot in zip(keep[1:], spill_slots):
                ssi = slot.sync_info
                if ssi is None:
                    ssi = bass_rust.SyncInfo(on_wait=[], on_update=[])
                    slot.sync_info = ssi
                ssi.on_wait = [w]
            keep = keep[:1]
        si.on_wait = keep


def _get_program():
    if "nc" not in _PROGRAM_CACHE:
        _PROGRAM_CACHE["nc"] = _build_program()
    return _PROGRAM_CACHE["nc"]


def _prep_x(x):
    xt = np.ascontiguousarray(np.asarray(x, np.float32).transpose(0, 2, 1))
    xp = np.zeros((B, C, LP), np.float16)
    xp[:, :, PAD : PAD + L] = xt.astype(np.float16)
    return [
        xp[i * BPC : (i + 1) * BPC].reshape(BPC * C, LP) for i in range(NCORES)
    ]


def kernel(
    x,
    kernels,
    channel_masks,
    bias_matrices,
    feature_mean,
    feature_std,
    _trace=False,
    _sim=False,
):
    wTv, wPv, cpkv, perms = _host_constants(
        kernels, channel_masks, bias_matrices, feature_mean, feature_std
    )
    x_slices = _prep_x(x)
    nc = _get_program()

    in_maps = [
        {"xT": x_slices[i], "wT": wTv, "wP": wPv, "cpk": cpkv}
        for i in range(NCORES)
    ]

    if _sim:
        import concourse.bass_interp as bass_interp

        try:
            nc.detect_race_conditions = False
        except Exception:
            pass
        sim = bass_interp.MultiCoreSim(nc, 1)
        sim.cores[0].assign_tensors(in_maps[0])
        sim.simulate()
        dev_outs = [np.array(sim.cores[0].tensor("out"))]
        full = np.zeros((B, 1344), np.float32)
        _scatter(full[:BPC], dev_outs[0], perms)
        _PROGRAM_CACHE["exec_time_ns"] = None
        return full

    if _trace:
        _install_ntff_hook_shim()

    from concourse.bass_utils import run_bass_kernel_spmd

    res = run_bass_kernel_spmd(
        nc,
        in_maps,
        core_ids=list(range(NCORES)),
        trace=_trace,
        trace_cores=list(range(NCORES)) if _trace else None,
    )
    _PROGRAM_CACHE["exec_time_ns"] = res.exec_time_ns
    _PROGRAM_CACHE["mean_exec_time_ns"] = res.mean_exec_time_ns
    _PROGRAM_CACHE["trace"] = res.instructions_and_trace

    full = np.empty((B, 1344), np.float32)
    for i in range(NCORES):
        _scatter(full[i * BPC : (i + 1) * BPC], res.results[i]["out"], perms)
    return full


def _install_ntff_hook_shim():
    import sys as _sys
    import types

    try:
        from antenv.axon_hooks import get_axon_ntff_profile_hook  # noqa: F401

        return
    except ImportError:
        pass
    from trn_agent_boot.trn_boot import _ntff_profile_via_ctypes

    hook = _ntff_profile_via_ctypes("/opt/axon/libaxon_pjrt.so")
    mod = types.ModuleType("antenv.axon_hooks")
    mod.get_axon_ntff_profile_hook = lambda: hook
    mod.set_axon_ntff_profile_hook = lambda h: None
    _sys.modules["antenv.axon_hooks"] = mod


def _scatter(dst, dev_out, perms):
    dev = np.asarray(dev_out, np.float32).reshape(BPC, K, D, F)
    fidx = np.arange(F)[None, :]
    for d_idx in range(D):
        cols = d_idx * (K * F) + perms[d_idx][:, None] * F + fidx
        dst[:, cols] = dev[:, :, d_idx, :]


# revision 6
# speedup vs baseline: 1.0865x; 1.0865x over previous
"""MiniRocket feature extraction kernel for Trainium2 (8 NeuronCores, data parallel).

v2: PSUM-direct counting. The baseline evicted resp PSUM->SBUF fp16 on ACT and
counted on DVE(is_gt)/ACT(Sign); measured HW shows DVE runs these at 1x
(1.04 ns/col) regardless of dtype, so the fp16 eviction bought nothing. Here:

  - resp [84, 2048] f32 stays in PSUM; counting ops read PSUM directly.
  - The odd-parity edge "poison" is folded into the matmul: patch row 72 is an
    edge-indicator row (1.0 on [0,pad) u [L-pad,L)) and W row 72 is -10000 for
    the 42 odd-parity kernels, so their edge responses can never exceed a bias.
    No per-iteration poison op anywhere.
  - Patch DMAs ride the SP HWDGE queue (SP sequencer is otherwise idle).
  - Counting is split DVE/ACT by a static schedule (l-ranges per feature)
    balanced against measured per-engine rates.
  - Counts accumulate into per-engine tables cnt_dve/cnt_act [84, 128]
    (col = b*16 + d*4 + f); one batched affine tail produces the output.

walrus encodes at most ONE sync wait per compute/DMA instruction;
_legalize_sync_waits (rules as in the baseline: self-wait drop, Ldweights
hoist, tail-drain pruning, same-sem max-collapse) legalizes Tile's waits.
"""

import os
import sys

for _p in (
    "/root/.axon_site",
    "/root/.axon_site/_ro/trn_rl_repo",
    "/root/.axon_site/_ro/pypackages",
    "/opt/trn_rl_repo",
):
    if os.path.isdir(_p) and _p not in sys.path:
        sys.path.append(_p)

import numpy as np

B, L, C = 64, 2048, 8
DILATIONS = (1, 2, 4, 8)
D = 4
K = 84
F = 4
KERNEL_LEN = 9
NCORES = 8
BPC = B // NCORES
PAD = 32
LP = L + 2 * PAD
POISON = -10000.0
BIG = 1.0e30
SAT8 = 448.0  # float8e4 (e4m3) saturation value

# Counting schedule: per (d) iteration, ops as (engine, f, lo, hi).
# 'dve' = tensor_scalar is_gt + accum (raw = count over [lo,hi))
# 'act' = Sign + accum (raw = 2*count - len, zeros counting 0)
# Measured HW rates: DVE accum-op ~2341ns per 2048 cols (accum_out disables
# the DVE fast modes); ACT Sign ~2000+185ns. Balanced split: DVE f0+f1,
# ACT f2+f3, with X1 trimming f1's tail over to ACT if < L.
ACT_MODE = "sign"
CV = 1024  # DVE counts cols [0,CV) of every feature; ACT counts [CV,L)

_PROGRAM_CACHE: dict = {}


def _schedule():
    ops = []
    for f in range(F):
        ops.append(("dve", f, 0, CV))
    for f in range(F):
        ops.append(("act", f, CV, L))
    return ops


def _parity_perm(d_idx: int) -> np.ndarray:
    k = np.arange(K)
    parity = (d_idx + k) % 2
    return np.concatenate([k[parity == 1], k[parity == 0]])


def _host_constants(kernels, channel_masks, bias_matrices, feature_mean, feature_std):
    """Build wT [73, 4*84] f16, eT [4, L] f16, cpk [84, 416] f32, perms."""
    kernels = np.asarray(kernels, np.float32)
    channel_masks = np.asarray(channel_masks, np.float32)
    bias_matrices = np.asarray(bias_matrices, np.float32)
    feature_mean = np.asarray(feature_mean, np.float32).reshape(D, K, F)
    feature_std = np.asarray(feature_std, np.float32).reshape(D, K, F)

    sched = _schedule()

    wT_blocks = []
    eT = np.zeros((D, L), np.float16)
    bias_dve = np.zeros((K, 16), np.float32)
    nbias_act = np.zeros((K, 16), np.float32)
    Ad = np.zeros((K, 128), np.float32)
    Aa = np.zeros((K, 128), np.float32)
    Bt = np.zeros((K, 128), np.float32)
    perms = []
    for d_idx, dil in enumerate(DILATIONS):
        perm = _parity_perm(d_idx)
        perms.append(perm)
        w = channel_masks[d_idx][perm][:, :, None] * kernels[perm][:, None, :]
        w73 = np.zeros((73, K), np.float32)
        w73[:72] = w.reshape(K, C * KERNEL_LEN).T
        w73[72, 0:42] = POISON  # odd-parity (trimmed) kernels come first
        wT_blocks.append(w73.astype(np.float16))

        pad = 4 * dil
        eT[d_idx, 0:pad] = 1.0
        eT[d_idx, L - pad : L] = 1.0

        w_sel = np.where(np.arange(K) < 42, 1.0 / (L - 2 * pad), 1.0 / L).astype(
            np.float32
        )
        bias_p = bias_matrices[d_idx][perm]  # [84, 4]
        mean_p = feature_mean[d_idx][perm]
        std_p = feature_std[d_idx][perm]
        A = w_sel[:, None] / std_p  # [84, 4]
        B0 = -mean_p / std_p  # [84, 4]

        for f in range(F):
            bias_dve[:, 4 * d_idx + f] = bias_p[:, f]
            nbias_act[:, 4 * d_idx + f] = (
                -BIG * bias_p[:, f] if ACT_MODE == "sat8" else -bias_p[:, f]
            )
        for b in range(BPC):
            for f in range(F):
                col = b * 16 + 4 * d_idx + f
                Bt[:, col] = B0[:, f]
                for eng, ff, lo, hi in sched:
                    if ff != f:
                        continue
                    if eng == "dve":
                        Ad[:, col] = A[:, f]
                    else:
                        if ACT_MODE == "sat8":
                            Aa[:, col] = A[:, f] / (2.0 * SAT8)
                        else:
                            Aa[:, col] = A[:, f] / 2.0
                        Bt[:, col] += A[:, f] * (hi - lo) / 2.0

    wT = np.concatenate(wT_blocks, axis=1)  # [73, 336]
    cpk = np.concatenate([bias_dve, nbias_act, Ad, Aa, Bt], axis=1)  # [84, 416]
    return wT[:72], wT[72:73].copy(), cpk, perms


def _build_program():
    from contextlib import ExitStack

    import bass_rust
    import concourse.bass as bass
    import concourse.tile as tile
    from concourse import mybir

    def shifted_ap(center_slice, dil):
        c = center_slice.copy()
        c.offset = c.offset - 4 * dil
        c.ap = bass_rust.VecI64Pair([[LP, C], [dil, KERNEL_LEN], [1, L]])
        return c

    f16 = mybir.dt.float16
    f32 = mybir.dt.float32
    f8 = mybir.dt.float8e4
    A_ = mybir.AluOpType
    AF = mybir.ActivationFunctionType

    sched = _schedule()

    nc = bass.Bass()
    xT = nc.declare_dram_parameter("xT", [BPC * C, LP], f16, isOutput=False)
    wT = nc.declare_dram_parameter("wT", [72, D * K], f16, isOutput=False)
    wP = nc.declare_dram_parameter("wP", [1, D * K], f16, isOutput=False)
    cpk = nc.declare_dram_parameter("cpk", [K, 416], f32, isOutput=False)
    out = nc.declare_dram_parameter("out", [BPC, K, 16], f32, isOutput=True)

    with tile.TileContext(nc) as tc, ExitStack() as ctx:
        cpool = ctx.enter_context(tc.tile_pool(name="const", bufs=1))
        patch_pool = ctx.enter_context(tc.tile_pool(name="patch", bufs=BPC * D))
        psum_lo = ctx.enter_context(tc.tile_pool(name="pslo", bufs=2, space="PSUM"))
        psum_hi = ctx.enter_context(tc.tile_pool(name="pshi", bufs=2, space="PSUM"))
        tr_pool = ctx.enter_context(tc.tile_pool(name="tr", bufs=1))
        cnt_pool = ctx.enter_context(tc.tile_pool(name="cnt", bufs=1))
        osb_pool = ctx.enter_context(tc.tile_pool(name="osb", bufs=1))
        scr_pool = ctx.enter_context(tc.tile_pool(name="scr", bufs=1))

        xsb = cpool.tile([BPC * C, LP], f16)
        nc.sync.dma_start(xsb[:], xT.ap())
        wsb = cpool.tile([72, D * K], f16)
        nc.sync.dma_start(wsb[:], wT.ap())
        wpois = cpool.tile([1, D * K], f16)
        nc.sync.dma_start(wpois[:], wP.ap())
        csb = cpool.tile([K, 416], f32)
        nc.sync.dma_start(csb[:], cpk.ap())

        trash_v = tr_pool.tile([K, L], f16)
        trash_a = tr_pool.tile([K, L], f8 if ACT_MODE == "sat8" else f16)
        cnt_dve = cnt_pool.tile([K, 128], f32)
        cnt_act = cnt_pool.tile([K, 128], f32)
        m1 = osb_pool.tile([K, 128], f32)
        m2 = osb_pool.tile([K, 128], f32)
        osb = osb_pool.tile([K, BPC * 16], f32)
        scr = scr_pool.tile([1, 72], f32)

        # zero the count tables on their own engines (engine-local FIFO later)
        nc.vector.memset(cnt_dve[:], 0.0)
        nc.scalar.memzero(cnt_act[:])

        ones = cpool.tile([1, 64], f16)
        nc.gpsimd.memset(ones[:], 1.0)

        # prime each engine's vector clock with the csb DMA tick; also pull
        # DVE's startup ticks into ACT's clock
        nc.vector.tensor_copy(m1[:, 0:1], csb[:, 0:1])
        nc.scalar.activation(scr[0:1, 0:1], csb[0:1, 0:1], AF.Copy)
        nc.scalar.activation(scr[0:1, 1:2], m1[0:1, 0:1], AF.Copy)

        for b in range(BPC):
            for d_idx, dil in enumerate(DILATIONS):
                it = b * D + d_idx
                pad = 4 * dil
                patch = patch_pool.tile([72, L], f16)
                nc.sync.dma_start(
                    patch[0:72, 0:L],
                    shifted_ap(xsb[C * b : C * b + C, PAD : PAD + L], dil),
                )
                # Two PSUM tiles per iteration: ps_lo is read ONLY by DVE,
                # ps_hi ONLY by ACT (Tile serializes cross-engine co-readers
                # of one PSUM tile, so each engine gets its own). Single
                # reader also keeps every matmul wait-set at 2 (patch DMA +
                # that engine's last count op), legal via the Ldweights hoist.
                ps_lo = psum_lo.tile([K, CV], f32)
                ps_hi = psum_hi.tile([K, L - CV], f32)
                for nt in range(4):
                    dst = (
                        ps_lo[:, nt * 512 : (nt + 1) * 512]
                        if nt * 512 < CV
                        else ps_hi[:, nt * 512 - CV : (nt + 1) * 512 - CV]
                    )
                    nc.tensor.matmul(
                        dst,
                        lhsT=wsb[:, d_idx * K : (d_idx + 1) * K],
                        rhs=patch[:, nt * 512 : (nt + 1) * 512],
                        start=True,
                        stop=True,
                    )
                # edge poison: accumulate W row 72 (-10000 on the 42 trimmed
                # kernels) against a ones row over the pad-wide edge columns,
                # so those responses can never exceed a bias.
                for dst in (ps_lo[:, 0:pad], ps_hi[:, L - CV - pad : L - CV]):
                    nc.tensor.matmul(
                        dst,
                        lhsT=wpois[0:1, d_idx * K : (d_idx + 1) * K],
                        rhs=ones[0:1, 0:pad],
                        start=False,
                        stop=True,
                        skip_group_check=True,
                    )
                for eng, f, lo, hi in sched:
                    col = b * 16 + 4 * d_idx + f
                    if eng == "dve":
                        nc.vector.tensor_scalar(
                            trash_v[:, lo:hi],
                            ps_lo[:, 0:CV],
                            csb[:, 4 * d_idx + f : 4 * d_idx + f + 1],
                            None,
                            A_.is_gt,
                            A_.add,
                            accum_out=cnt_dve[:, col : col + 1],
                        )
                    else:
                        nc.scalar.activation(
                            trash_a[:, lo:hi],
                            ps_hi[:, 0 : L - CV],
                            AF.Sign,
                            bias=csb[:, 16 + 4 * d_idx + f : 16 + 4 * d_idx + f + 1],
                            accum_out=cnt_act[:, col : col + 1],
                        )

        # tail: osb = cnt_dve*Ad + cnt_act*Aa + Bt  (all [84, 128])
        nc.vector.tensor_tensor(m1[:], cnt_dve[:], csb[:, 32:160], A_.mult)
        nc.vector.tensor_tensor(m2[:], cnt_act[:], csb[:, 160:288], A_.mult)
        nc.vector.tensor_tensor(m1[:], m1[:], m2[:], A_.add)
        nc.vector.tensor_tensor(osb[:], m1[:], csb[:, 288:416], A_.add)

        dst = out.ap().copy()
        dst.ap = bass_rust.VecI64Pair([[16, K], [K * 16, BPC], [1, 16]])
        nc.sync.dma_start(dst, osb[:])

    _legalize_sync_waits(nc, bass_rust)
    return nc


_FIFO_SELF_SEM = {
    "Matmult": "PE_",
    "Ldweights": "PE_",
    "Activation": "Activation_",
    "TensorScalarPtr": "DVE_",
    "TensorTensor": "DVE_",
    "TensorReduce": "DVE_",
    "Memset": None,
}


def _legalize_sync_waits(nc, bass_rust):
    """walrus encodes at most ONE sync wait per compute/DMA instruction.
    Rules (validated in CoreSim + hardware by the baseline kernel):
     0. Collapse multiple waits on the same semaphore to the max tick.
     1. Drop same-engine self-waits when an instruction holds other waits.
     2. Hoist extra Matmult waits onto the immediately-preceding Ldweights.
     3. Prune the kernel-tail SP drain (see baseline docstring).
    """
    blocks = list(nc.m.functions[0].blocks)
    end_blk = next(b for b in blocks if b.name.endswith("_end"))

    max_waited: dict = {}
    for blk in blocks:
        for inst in blk.instructions:
            if blk is end_blk:
                continue
            si = inst.sync_info
            for w in si.on_wait if si and si.on_wait else []:
                if w.wait_value > max_waited.get(w.ant_name, -1):
                    max_waited[w.ant_name] = w.wait_value

    def _tensor_of(arg):
        try:
            return arg.memsetref or ""
        except AttributeError:
            return ""

    # patch-feeding DMAs (dst tile "patch*", src "xsb*"), in iteration order
    patch_dmas = []
    for blk in blocks:
        for inst in blk.instructions:
            if inst.opcode == "DMACopy" and inst.outs and inst.ins:
                if _tensor_of(inst.outs[0]).startswith("patch") and _tensor_of(
                    inst.ins[0]
                ).startswith("xsb"):
                    patch_dmas.append(inst)
    for blk in blocks:
        prev = None
        proc: list = []  # processed instructions in this block, in order
        for inst in blk.instructions:
            si = inst.sync_info
            if si is None or not si.on_wait:
                prev = inst
                proc.append(inst)
                continue
            waits = list(si.on_wait)
            # rule 0: same-sem collapse
            best: dict = {}
            for w in waits:
                if w.ant_name not in best or w.wait_value > best[w.ant_name].wait_value:
                    best[w.ant_name] = w
            waits = list(best.values())
            pfx = _FIFO_SELF_SEM.get(inst.opcode)
            if pfx and len(waits) > 1:
                waits = [w for w in waits if not w.ant_name.startswith(pfx)]
            if inst.opcode == "Matmult" and len(waits) > 1:
                # The ACT wait is the closer op whose vector clock (via the
                # gate) covers the DVE readers of the PSUM slot.
                if any(w.ant_name.startswith("Activation_") for w in waits):
                    waits = [w for w in waits if not w.ant_name.startswith("DVE_")]
            if inst.opcode == "DMACopy" and len(waits) > 1:
                # queue FIFO already orders DMAs on the same queue
                non_q = [w for w in waits if not w.ant_name.startswith("DMAHW")]
                if non_q:
                    waits = non_q
            if (
                inst.opcode in ("Activation", "TensorScalarPtr", "TensorTensor")
                and len(waits) > 1
            ):
                # backward-deposit: park excess waits on earlier same-engine
                # instructions with empty wait sets (engine FIFO preserves the
                # ordering; producers of these waits are upstream of this
                # engine's stream, so no cycles).
                eng = inst.engine
                for pi in reversed(proc):
                    if len(waits) <= 1:
                        break
                    if pi.engine != eng or pi.opcode in ("Drain",):
                        continue
                    psi2 = pi.sync_info
                    if psi2 is not None and psi2.on_wait:
                        continue
                    if psi2 is None:
                        psi2 = bass_rust.SyncInfo(on_wait=[], on_update=[])
                        pi.sync_info = psi2
                    psi2.on_wait = [waits.pop(0)]
                assert len(waits) <= 1, (
                    f"{inst.opcode} {inst.name}: could not legalize "
                    f"{len(waits)} waits"
                )
            if inst.opcode == "Matmult" and len(waits) > 1:
                assert prev is not None and prev.opcode == "Ldweights", (
                    f"matmul {inst.name} has {len(waits)} waits and no "
                    f"preceding Ldweights (prev={prev and prev.opcode})"
                )
                psi = prev.sync_info
                if psi is None:
                    psi = bass_rust.SyncInfo(on_wait=[], on_update=[])
                    prev.sync_info = psi
                psi.on_wait = list(psi.on_wait) + waits[:-1]
                waits = waits[-1:]
            si.on_wait = waits
            prev = inst
            proc.append(inst)

    end_insts = list(end_blk.instructions)
    tail = end_insts[0]
    assert tail.opcode == "Drain", f"unexpected end block head {tail.opcode}"
    si = tail.sync_info
    if si and len(si.on_wait) > 1:
        eng_pfx = ("Activation_", "PE_", "DVE_", "Pool_", "SP_")
        keep = [
            w
            for w in si.on_wait
            if not w.ant_name.startswith(eng_pfx)
            and max_waited.get(w.ant_name, -1) < w.wait_value
        ]
        if len(keep) > 1:
            spill_slots = []
            for inst in end_insts[1:]:
                if inst.opcode == "ISA":
                    break
                isi = inst.sync_info
                if inst.opcode == "Drain" and (not isi or not isi.on_wait):
                    spill_slots.append(inst)
            assert len(spill_slots) >= len(keep) - 1, (
                f"tail drain needs {len(keep)} wait slots, "
                f"only {1 + len(spill_slots)} available"
            )
            for w, slot in zip(keep[1:], spill_slots):
                ssi = slot.sync_info
                if ssi is None:
                    ssi = bass_rust.SyncInfo(on_wait=[], on_update=[])
                    slot.sync_info = ssi
                ssi.on_wait = [w]
            keep = keep[:1]
        si.on_wait = keep


def _get_program():
    if "nc" not in _PROGRAM_CACHE:
        _PROGRAM_CACHE["nc"] = _build_program()
    return _PROGRAM_CACHE["nc"]


def _prep_x(x):
    xt = np.ascontiguousarray(np.asarray(x, np.float32).transpose(0, 2, 1))
    xp = np.zeros((B, C, LP), np.float16)
    xp[:, :, PAD : PAD + L] = xt.astype(np.float16)
    return [
        xp[i * BPC : (i + 1) * BPC].reshape(BPC * C, LP) for i in range(NCORES)
    ]


def kernel(
    x,
    kernels,
    channel_masks,
    bias_matrices,
    feature_mean,
    feature_std,
    _trace=False,
    _sim=False,
):
    wTv, wPv, cpkv, perms = _host_constants(
        kernels, channel_masks, bias_matrices, feature_mean, feature_std
    )
    x_slices = _prep_x(x)
    nc = _get_program()

    in_maps = [
        {"xT": x_slices[i], "wT": wTv, "wP": wPv, "cpk": cpkv}
        for i in range(NCORES)
    ]

    if _sim:
        import concourse.bass_interp as bass_interp

        try:
            nc.detect_race_conditions = False
        except Exception:
            pass
        sim = bass_interp.MultiCoreSim(nc, 1)
        sim.cores[0].assign_tensors(in_maps[0])
        sim.simulate()
        dev_outs = [np.array(sim.cores[0].tensor("out"))]
        full = np.zeros((B, 1344), np.float32)
        _scatter(full[:BPC], dev_outs[0], perms)
        _PROGRAM_CACHE["exec_time_ns"] = None
        return full

    if _trace:
        _install_ntff_hook_shim()

    from concourse.bass_utils import run_bass_kernel_spmd

    res = run_bass_kernel_spmd(
        nc,
        in_maps,
        core_ids=list(range(NCORES)),
        trace=_trace,
        trace_cores=list(range(NCORES)) if _trace else None,
    )
    _PROGRAM_CACHE["exec_time_ns"] = res.exec_time_ns
    _PROGRAM_CACHE["mean_exec_time_ns"] = res.mean_exec_time_ns
    _PROGRAM_CACHE["trace"] = res.instructions_and_trace

    full = np.empty((B, 1344), np.float32)
    for i in range(NCORES):
        _scatter(full[i * BPC : (i + 1) * BPC], res.results[i]["out"], perms)
    return full


def _install_ntff_hook_shim():
    import sys as _sys
    import types

    try:
        from antenv.axon_hooks import get_axon_ntff_profile_hook  # noqa: F401

        return
    except ImportError:
        pass
    from trn_agent_boot.trn_boot import _ntff_profile_via_ctypes

    hook = _ntff_profile_via_ctypes("/opt/axon/libaxon_pjrt.so")
    mod = types.ModuleType("antenv.axon_hooks")
    mod.get_axon_ntff_profile_hook = lambda: hook
    mod.set_axon_ntff_profile_hook = lambda h: None
    _sys.modules["antenv.axon_hooks"] = mod


def _scatter(dst, dev_out, perms):
    dev = np.asarray(dev_out, np.float32).reshape(BPC, K, D, F)
    fidx = np.arange(F)[None, :]
    for d_idx in range(D):
        cols = d_idx * (K * F) + perms[d_idx][:, None] * F + fidx
        dst[:, cols] = dev[:, :, d_idx, :]


# revision 7
# speedup vs baseline: 1.0867x; 1.0002x over previous
"""MiniRocket feature extraction kernel for Trainium2 (8 NeuronCores, data parallel).

v2: PSUM-direct counting. The baseline evicted resp PSUM->SBUF fp16 on ACT and
counted on DVE(is_gt)/ACT(Sign); measured HW shows DVE runs these at 1x
(1.04 ns/col) regardless of dtype, so the fp16 eviction bought nothing. Here:

  - resp [84, 2048] f32 stays in PSUM; counting ops read PSUM directly.
  - The odd-parity edge "poison" is folded into the matmul: patch row 72 is an
    edge-indicator row (1.0 on [0,pad) u [L-pad,L)) and W row 72 is -10000 for
    the 42 odd-parity kernels, so their edge responses can never exceed a bias.
    No per-iteration poison op anywhere.
  - Patch DMAs ride the SP HWDGE queue (SP sequencer is otherwise idle).
  - Counting is split DVE/ACT by a static schedule (l-ranges per feature)
    balanced against measured per-engine rates.
  - Counts accumulate into per-engine tables cnt_dve/cnt_act [84, 128]
    (col = b*16 + d*4 + f); one batched affine tail produces the output.

walrus encodes at most ONE sync wait per compute/DMA instruction;
_legalize_sync_waits (rules as in the baseline: self-wait drop, Ldweights
hoist, tail-drain pruning, same-sem max-collapse) legalizes Tile's waits.
"""

import os
import sys

for _p in (
    "/root/.axon_site",
    "/root/.axon_site/_ro/trn_rl_repo",
    "/root/.axon_site/_ro/pypackages",
    "/opt/trn_rl_repo",
):
    if os.path.isdir(_p) and _p not in sys.path:
        sys.path.append(_p)

import numpy as np

B, L, C = 64, 2048, 8
DILATIONS = (1, 2, 4, 8)
D = 4
K = 84
F = 4
KERNEL_LEN = 9
NCORES = 8
BPC = B // NCORES
PAD = 32
LP = L + 2 * PAD
POISON = -10000.0
BIG = 1.0e30
SAT8 = 448.0  # float8e4 (e4m3) saturation value

# Counting schedule: per (d) iteration, ops as (engine, f, lo, hi).
# 'dve' = tensor_scalar is_gt + accum (raw = count over [lo,hi))
# 'act' = Sign + accum (raw = 2*count - len, zeros counting 0)
# Measured HW rates: DVE accum-op ~2341ns per 2048 cols (accum_out disables
# the DVE fast modes); ACT Sign ~2000+185ns. Balanced split: DVE f0+f1,
# ACT f2+f3, with X1 trimming f1's tail over to ACT if < L.
ACT_MODE = "sign"
CV = 1024  # DVE counts cols [0,CV) of every feature; ACT counts [CV,L)

_PROGRAM_CACHE: dict = {}


def _schedule():
    ops = []
    for f in range(F):
        ops.append(("dve", f, 0, CV))
    for f in range(F):
        ops.append(("act", f, CV, L))
    return ops


def _parity_perm(d_idx: int) -> np.ndarray:
    k = np.arange(K)
    parity = (d_idx + k) % 2
    return np.concatenate([k[parity == 1], k[parity == 0]])


def _host_constants(kernels, channel_masks, bias_matrices, feature_mean, feature_std):
    """Build wT [73, 4*84] f16, eT [4, L] f16, cpk [84, 416] f32, perms."""
    kernels = np.asarray(kernels, np.float32)
    channel_masks = np.asarray(channel_masks, np.float32)
    bias_matrices = np.asarray(bias_matrices, np.float32)
    feature_mean = np.asarray(feature_mean, np.float32).reshape(D, K, F)
    feature_std = np.asarray(feature_std, np.float32).reshape(D, K, F)

    sched = _schedule()

    wT_blocks = []
    eT = np.zeros((D, L), np.float16)
    bias_dve = np.zeros((K, 16), np.float32)
    nbias_act = np.zeros((K, 16), np.float32)
    Ad = np.zeros((K, 128), np.float32)
    Aa = np.zeros((K, 128), np.float32)
    Bt = np.zeros((K, 128), np.float32)
    perms = []
    for d_idx, dil in enumerate(DILATIONS):
        perm = _parity_perm(d_idx)
        perms.append(perm)
        w = channel_masks[d_idx][perm][:, :, None] * kernels[perm][:, None, :]
        w73 = np.zeros((73, K), np.float32)
        w73[:72] = w.reshape(K, C * KERNEL_LEN).T
        w73[72, 0:42] = POISON  # odd-parity (trimmed) kernels come first
        wT_blocks.append(w73.astype(np.float16))

        pad = 4 * dil
        eT[d_idx, 0:pad] = 1.0
        eT[d_idx, L - pad : L] = 1.0

        w_sel = np.where(np.arange(K) < 42, 1.0 / (L - 2 * pad), 1.0 / L).astype(
            np.float32
        )
        bias_p = bias_matrices[d_idx][perm]  # [84, 4]
        mean_p = feature_mean[d_idx][perm]
        std_p = feature_std[d_idx][perm]
        A = w_sel[:, None] / std_p  # [84, 4]
        B0 = -mean_p / std_p  # [84, 4]

        for f in range(F):
            bias_dve[:, 4 * d_idx + f] = bias_p[:, f]
            nbias_act[:, 4 * d_idx + f] = (
                -BIG * bias_p[:, f] if ACT_MODE == "sat8" else -bias_p[:, f]
            )
        for b in range(BPC):
            for f in range(F):
                col = b * 16 + 4 * d_idx + f
                Bt[:, col] = B0[:, f]
                for eng, ff, lo, hi in sched:
                    if ff != f:
                        continue
                    if eng == "dve":
                        Ad[:, col] = A[:, f]
                    else:
                        if ACT_MODE == "sat8":
                            Aa[:, col] = A[:, f] / (2.0 * SAT8)
                        else:
                            Aa[:, col] = A[:, f] / 2.0
                        Bt[:, col] += A[:, f] * (hi - lo) / 2.0

    wT = np.concatenate(wT_blocks, axis=1)  # [73, 336]
    cpk = np.concatenate([bias_dve, nbias_act, Ad, Aa, Bt], axis=1)  # [84, 416]
    return wT[:72], wT[72:73].copy(), cpk, perms


def _build_program():
    from contextlib import ExitStack

    import bass_rust
    import concourse.bass as bass
    import concourse.tile as tile
    from concourse import mybir

    def shifted_ap(center_slice, dil):
        c = center_slice.copy()
        c.offset = c.offset - 4 * dil
        c.ap = bass_rust.VecI64Pair([[LP, C], [dil, KERNEL_LEN], [1, L]])
        return c

    f16 = mybir.dt.float16
    f32 = mybir.dt.float32
    f8 = mybir.dt.float8e4
    A_ = mybir.AluOpType
    AF = mybir.ActivationFunctionType

    sched = _schedule()

    nc = bass.Bass()
    xT = nc.declare_dram_parameter("xT", [BPC * C, LP], f16, isOutput=False)
    wT = nc.declare_dram_parameter("wT", [72, D * K], f16, isOutput=False)
    wP = nc.declare_dram_parameter("wP", [1, D * K], f16, isOutput=False)
    cpk = nc.declare_dram_parameter("cpk", [K, 416], f32, isOutput=False)
    out = nc.declare_dram_parameter("out", [BPC, K, 16], f32, isOutput=True)

    with tile.TileContext(nc) as tc, ExitStack() as ctx:
        cpool = ctx.enter_context(tc.tile_pool(name="const", bufs=1))
        patch_pool = ctx.enter_context(tc.tile_pool(name="patch", bufs=BPC * D))
        psum_lo = ctx.enter_context(tc.tile_pool(name="pslo", bufs=2, space="PSUM"))
        psum_hi = ctx.enter_context(tc.tile_pool(name="pshi", bufs=2, space="PSUM"))
        tr_pool = ctx.enter_context(tc.tile_pool(name="tr", bufs=1))
        cnt_pool = ctx.enter_context(tc.tile_pool(name="cnt", bufs=1))
        osb_pool = ctx.enter_context(tc.tile_pool(name="osb", bufs=1))
        scr_pool = ctx.enter_context(tc.tile_pool(name="scr", bufs=1))

        xsb = cpool.tile([BPC * C, LP], f16)
        nc.sync.dma_start(xsb[:], xT.ap())
        wsb = cpool.tile([72, D * K], f16)
        nc.sync.dma_start(wsb[:], wT.ap())
        wpois = cpool.tile([1, D * K], f16)
        nc.sync.dma_start(wpois[:], wP.ap())
        csb = cpool.tile([K, 416], f32)
        nc.sync.dma_start(csb[:], cpk.ap())

        trash_v = tr_pool.tile([K, L], f16)
        trash_a = tr_pool.tile([K, L], f8 if ACT_MODE == "sat8" else f16)
        cnt_dve = cnt_pool.tile([K, 128], f32)
        cnt_act = cnt_pool.tile([K, 128], f32)
        m1 = osb_pool.tile([K, 128], f32)
        m2 = osb_pool.tile([K, 128], f32)
        osb = osb_pool.tile([K, BPC * 16], f32)
        scr = scr_pool.tile([1, 72], f32)

        # zero the count tables on their own engines (engine-local FIFO later)
        nc.vector.memset(cnt_dve[:], 0.0)
        nc.scalar.memzero(cnt_act[:])

        ones = cpool.tile([1, 64], f16)
        nc.gpsimd.memset(ones[:], 1.0)

        # prime each engine's vector clock with the csb DMA tick; also pull
        # DVE's startup ticks into ACT's clock
        nc.vector.tensor_copy(m1[:, 0:1], csb[:, 0:1])
        nc.scalar.activation(scr[0:1, 0:1], csb[0:1, 0:1], AF.Copy)
        nc.scalar.activation(scr[0:1, 1:2], m1[0:1, 0:1], AF.Copy)

        for b in range(BPC):
            for d_idx, dil in enumerate(DILATIONS):
                it = b * D + d_idx
                pad = 4 * dil
                patch = patch_pool.tile([72, L], f16)
                nc.sync.dma_start(
                    patch[0:72, 0:L],
                    shifted_ap(xsb[C * b : C * b + C, PAD : PAD + L], dil),
                )
                # Two PSUM tiles per iteration: ps_lo is read ONLY by DVE,
                # ps_hi ONLY by ACT (Tile serializes cross-engine co-readers
                # of one PSUM tile, so each engine gets its own). Single
                # reader also keeps every matmul wait-set at 2 (patch DMA +
                # that engine's last count op), legal via the Ldweights hoist.
                ps_lo = psum_lo.tile([K, CV], f32)
                ps_hi = psum_hi.tile([K, L - CV], f32)
                def edge_poison(dst):
                    # accumulate W row 72 (-10000 on the 42 trimmed kernels)
                    # against a ones row over the pad-wide edge columns, so
                    # those responses can never exceed a bias.
                    nc.tensor.matmul(
                        dst,
                        lhsT=wpois[0:1, d_idx * K : (d_idx + 1) * K],
                        rhs=ones[0:1, 0:pad],
                        start=False,
                        stop=True,
                        skip_group_check=True,
                    )

                def mains(tile_ap, lo, hi):
                    # 512-wide blocks (PSUM bank-sized matmul outputs)
                    for c0 in range(lo, hi, 512):
                        c1 = min(c0 + 512, hi)
                        nc.tensor.matmul(
                            tile_ap[:, c0 - lo : c1 - lo],
                            lhsT=wsb[:, d_idx * K : (d_idx + 1) * K],
                            rhs=patch[:, c0:c1],
                            start=True,
                            stop=True,
                        )

                # lo tile completes first (its edge poison emitted right
                # after its mains), so DVE can start while the hi tile is
                # still being filled.
                mains(ps_lo, 0, CV)
                edge_poison(ps_lo[:, 0:pad])
                mains(ps_hi, CV, L)
                edge_poison(ps_hi[:, L - CV - pad : L - CV])
                for eng, f, lo, hi in sched:
                    col = b * 16 + 4 * d_idx + f
                    if eng == "dve":
                        nc.vector.tensor_scalar(
                            trash_v[:, lo:hi],
                            ps_lo[:, 0:CV],
                            csb[:, 4 * d_idx + f : 4 * d_idx + f + 1],
                            None,
                            A_.is_gt,
                            A_.add,
                            accum_out=cnt_dve[:, col : col + 1],
                        )
                    else:
                        nc.scalar.activation(
                            trash_a[:, lo:hi],
                            ps_hi[:, 0 : L - CV],
                            AF.Sign,
                            bias=csb[:, 16 + 4 * d_idx + f : 16 + 4 * d_idx + f + 1],
                            accum_out=cnt_act[:, col : col + 1],
                        )

        # tail: osb = cnt_dve*Ad + cnt_act*Aa + Bt  (all [84, 128])
        nc.vector.tensor_tensor(m1[:], cnt_dve[:], csb[:, 32:160], A_.mult)
        nc.vector.tensor_tensor(m2[:], cnt_act[:], csb[:, 160:288], A_.mult)
        nc.vector.tensor_tensor(m1[:], m1[:], m2[:], A_.add)
        nc.vector.tensor_tensor(osb[:], m1[:], csb[:, 288:416], A_.add)

        dst = out.ap().copy()
        dst.ap = bass_rust.VecI64Pair([[16, K], [K * 16, BPC], [1, 16]])
        nc.sync.dma_start(dst, osb[:])

    _legalize_sync_waits(nc, bass_rust)
    return nc


_FIFO_SELF_SEM = {
    "Matmult": "PE_",
    "Ldweights": "PE_",
    "Activation": "Activation_",
    "TensorScalarPtr": "DVE_",
    "TensorTensor": "DVE_",
    "TensorReduce": "DVE_",
    "Memset": None,
}


def _legalize_sync_waits(nc, bass_rust):
    """walrus encodes at most ONE sync wait per compute/DMA instruction.
    Rules (validated in CoreSim + hardware by the baseline kernel):
     0. Collapse multiple waits on the same semaphore to the max tick.
     1. Drop same-engine self-waits when an instruction holds other waits.
     2. Hoist extra Matmult waits onto the immediately-preceding Ldweights.
     3. Prune the kernel-tail SP drain (see baseline docstring).
    """
    blocks = list(nc.m.functions[0].blocks)
    end_blk = next(b for b in blocks if b.name.endswith("_end"))

    max_waited: dict = {}
    for blk in blocks:
        for inst in blk.instructions:
            if blk is end_blk:
                continue
            si = inst.sync_info
            for w in si.on_wait if si and si.on_wait else []:
                if w.wait_value > max_waited.get(w.ant_name, -1):
                    max_waited[w.ant_name] = w.wait_value

    def _tensor_of(arg):
        try:
            return arg.memsetref or ""
        except AttributeError:
            return ""

    # patch-feeding DMAs (dst tile "patch*", src "xsb*"), in iteration order
    patch_dmas = []
    for blk in blocks:
        for inst in blk.instructions:
            if inst.opcode == "DMACopy" and inst.outs and inst.ins:
                if _tensor_of(inst.outs[0]).startswith("patch") and _tensor_of(
                    inst.ins[0]
                ).startswith("xsb"):
                    patch_dmas.append(inst)
    for blk in blocks:
        prev = None
        proc: list = []  # processed instructions in this block, in order
        for inst in blk.instructions:
            si = inst.sync_info
            if si is None or not si.on_wait:
                prev = inst
                proc.append(inst)
                continue
            waits = list(si.on_wait)
            # rule 0: same-sem collapse
            best: dict = {}
            for w in waits:
                if w.ant_name not in best or w.wait_value > best[w.ant_name].wait_value:
                    best[w.ant_name] = w
            waits = list(best.values())
            pfx = _FIFO_SELF_SEM.get(inst.opcode)
            if pfx and len(waits) > 1:
                waits = [w for w in waits if not w.ant_name.startswith(pfx)]
            if inst.opcode == "Matmult" and len(waits) > 1:
                # The ACT wait is the closer op whose vector clock (via the
                # gate) covers the DVE readers of the PSUM slot.
                if any(w.ant_name.startswith("Activation_") for w in waits):
                    waits = [w for w in waits if not w.ant_name.startswith("DVE_")]
            if inst.opcode == "DMACopy" and len(waits) > 1:
                # queue FIFO already orders DMAs on the same queue
                non_q = [w for w in waits if not w.ant_name.startswith("DMAHW")]
                if non_q:
                    waits = non_q
            if (
                inst.opcode in ("Activation", "TensorScalarPtr", "TensorTensor")
                and len(waits) > 1
            ):
                # backward-deposit: park excess waits on earlier same-engine
                # instructions with empty wait sets (engine FIFO preserves the
                # ordering; producers of these waits are upstream of this
                # engine's stream, so no cycles).
                eng = inst.engine
                for pi in reversed(proc):
                    if len(waits) <= 1:
                        break
                    if pi.engine != eng or pi.opcode in ("Drain",):
                        continue
                    psi2 = pi.sync_info
                    if psi2 is not None and psi2.on_wait:
                        continue
                    if psi2 is None:
                        psi2 = bass_rust.SyncInfo(on_wait=[], on_update=[])
                        pi.sync_info = psi2
                    psi2.on_wait = [waits.pop(0)]
                assert len(waits) <= 1, (
                    f"{inst.opcode} {inst.name}: could not legalize "
                    f"{len(waits)} waits"
                )
            if inst.opcode == "Matmult" and len(waits) > 1:
                assert prev is not None and prev.opcode == "Ldweights", (
                    f"matmul {inst.name} has {len(waits)} waits and no "
                    f"preceding Ldweights (prev={prev and prev.opcode})"
                )
                psi = prev.sync_info
                if psi is None:
                    psi = bass_rust.SyncInfo(on_wait=[], on_update=[])
                    prev.sync_info = psi
                psi.on_wait = list(psi.on_wait) + waits[:-1]
                waits = waits[-1:]
            si.on_wait = waits
            prev = inst
            proc.append(inst)

    end_insts = list(end_blk.instructions)
    tail = end_insts[0]
    assert tail.opcode == "Drain", f"unexpected end block head {tail.opcode}"
    si = tail.sync_info
    if si and len(si.on_wait) > 1:
        eng_pfx = ("Activation_", "PE_", "DVE_", "Pool_", "SP_")
        keep = [
            w
            for w in si.on_wait
            if not w.ant_name.startswith(eng_pfx)
            and max_waited.get(w.ant_name, -1) < w.wait_value
        ]
        if len(keep) > 1:
            spill_slots = []
            for inst in end_insts[1:]:
                if inst.opcode == "ISA":
                    break
                isi = inst.sync_info
                if inst.opcode == "Drain" and (not isi or not isi.on_wait):
                    spill_slots.append(inst)
            assert len(spill_slots) >= len(keep) - 1, (
                f"tail drain needs {len(keep)} wait slots, "
                f"only {1 + len(spill_slots)} available"
            )
            for w, slot in zip(keep[1:], spill_slots):
                ssi = slot.sync_info
                if ssi is None:
                    ssi = bass_rust.SyncInfo(on_wait=[], on_update=[])
                    slot.sync_info = ssi
                ssi.on_wait = [w]
            keep = keep[:1]
        si.on_wait = keep


def _get_program():
    if "nc" not in _PROGRAM_CACHE:
        _PROGRAM_CACHE["nc"] = _build_program()
    return _PROGRAM_CACHE["nc"]


def _prep_x(x):
    xt = np.ascontiguousarray(np.asarray(x, np.float32).transpose(0, 2, 1))
    xp = np.zeros((B, C, LP), np.float16)
    xp[:, :, PAD : PAD + L] = xt.astype(np.float16)
    return [
        xp[i * BPC : (i + 1) * BPC].reshape(BPC * C, LP) for i in range(NCORES)
    ]


def kernel(
    x,
    kernels,
    channel_masks,
    bias_matrices,
    feature_mean,
    feature_std,
    _trace=False,
    _sim=False,
):
    wTv, wPv, cpkv, perms = _host_constants(
        kernels, channel_masks, bias_matrices, feature_mean, feature_std
    )
    x_slices = _prep_x(x)
    nc = _get_program()

    in_maps = [
        {"xT": x_slices[i], "wT": wTv, "wP": wPv, "cpk": cpkv}
        for i in range(NCORES)
    ]

    if _sim:
        import concourse.bass_interp as bass_interp

        try:
            nc.detect_race_conditions = False
        except Exception:
            pass
        sim = bass_interp.MultiCoreSim(nc, 1)
        sim.cores[0].assign_tensors(in_maps[0])
        sim.simulate()
        dev_outs = [np.array(sim.cores[0].tensor("out"))]
        full = np.zeros((B, 1344), np.float32)
        _scatter(full[:BPC], dev_outs[0], perms)
        _PROGRAM_CACHE["exec_time_ns"] = None
        return full

    if _trace:
        _install_ntff_hook_shim()

    from concourse.bass_utils import run_bass_kernel_spmd

    res = run_bass_kernel_spmd(
        nc,
        in_maps,
        core_ids=list(range(NCORES)),
        trace=_trace,
        trace_cores=list(range(NCORES)) if _trace else None,
    )
    _PROGRAM_CACHE["exec_time_ns"] = res.exec_time_ns
    _PROGRAM_CACHE["mean_exec_time_ns"] = res.mean_exec_time_ns
    _PROGRAM_CACHE["trace"] = res.instructions_and_trace

    full = np.empty((B, 1344), np.float32)
    for i in range(NCORES):
        _scatter(full[i * BPC : (i + 1) * BPC], res.results[i]["out"], perms)
    return full


def _install_ntff_hook_shim():
    import sys as _sys
    import types

    try:
        from antenv.axon_hooks import get_axon_ntff_profile_hook  # noqa: F401

        return
    except ImportError:
        pass
    from trn_agent_boot.trn_boot import _ntff_profile_via_ctypes

    hook = _ntff_profile_via_ctypes("/opt/axon/libaxon_pjrt.so")
    mod = types.ModuleType("antenv.axon_hooks")
    mod.get_axon_ntff_profile_hook = lambda: hook
    mod.set_axon_ntff_profile_hook = lambda h: None
    _sys.modules["antenv.axon_hooks"] = mod


def _scatter(dst, dev_out, perms):
    dev = np.asarray(dev_out, np.float32).reshape(BPC, K, D, F)
    fidx = np.arange(F)[None, :]
    for d_idx in range(D):
        cols = d_idx * (K * F) + perms[d_idx][:, None] * F + fidx
        dst[:, cols] = dev[:, :, d_idx, :]
